# revision 21
# baseline (speedup 1.0000x reference)
"""AttentionPooling kernel for 8 Trainium2 NeuronCores (fp8 rewrite).

Reference computation (per batch b):
    Q = x@Wq + bq; K = x@Wk + bk; V = x@Wv + bv
    out[b] = mean_q softmax(Q K^T / sqrt(H)) @ V

Math used to cut work:
  * bk drops (softmax row-shift invariance); bv adds at the end.
  * scores = Q'' x^T with Q'' = x M + 1 u^T, M = Wq Wk^T/16, u = bq Wk^T/16
    (M, u precomputed on host) -> K projection never computed.
  * mean over q: with E = exp(scores), r_q = 4096/rowsum(E),
        out = (1/(N*4096)) * w^T V + bv,  w[k] = sum_q r_q E[q,k]
    w accumulated on the PE as DoubleRow fp8 rank-1 passes (2 q-chunks/pass).
  * All big matmuls in fp8e4m3 DoubleRow (2x PE rate, 256-contraction/pass).
  * exp split: 3/4 of tiles on ScalarE (fp8 out + free accum row-sums);
    1/4 on VectorE via an e4m3 bit-trick (byte = trunc(8*log2e*s + 56)),
    row-sums for those via a VectorE reduce.

Sharding: batch b -> core b (8 cores, B=8), SPMD, no collectives.
"""

import os
import sys

import numpy as np

B, N, D, H = 8, 4096, 256, 256
NCORES = 8
NQ = N // 128          # 32 q-chunks
KT = 4                 # score sub-tiles per q-chunk ([128, 1024] each)
KSUB = N // KT         # 1024 columns per sub-tile (2 PSUM banks)
LOG2E = 1.4426950408889634
# f16 bit-trick: bits = trunc(1024*log2e*s + B); B calibrated so the
# piecewise-linear-mantissa exp is mean-centered (see kernel notes)
T16_SCALE = 1024.0 * LOG2E
T16_BIAS = 15302.0

for _p in (
    "/opt/trn_rl_repo",
    "/root/.axon_site",
    "/root/.axon_site/_ro/trn_rl_repo",
    "/root/.axon_site/_ro/pypackages",
):
    if os.path.isdir(_p) and _p not in sys.path:
        sys.path.append(_p)

_CACHE = {}


def _build_program():
    import concourse.tile as tile
    from concourse import bacc, bass_isa, masks, mybir

    dt = mybir.dt
    F32, F16, FP8 = dt.float32, dt.float16, dt.float8e4
    U8, U16 = dt.uint8, dt.uint16
    AF = mybir.ActivationFunctionType
    DR = mybir.MatmulPerfMode.DoubleRow
    ALU = mybir.AluOpType
    AX = mybir.AxisListType

    nc = bacc.Bacc("TRN2", target_bir_lowering=False, debug=False,
                   num_devices=NCORES)

    x_d = nc.dram_tensor("x8", [128, 2 * N], FP8, kind="ExternalInput").ap()
    x16_d = nc.dram_tensor("x16", [128, 2 * N], F16, kind="ExternalInput").ap()
    m_d = nc.dram_tensor("m8", [128, 2 * H], FP8, kind="ExternalInput").ap()
    wv_d = nc.dram_tensor("wv16", [128, 2 * H], F16, kind="ExternalInput").ap()
    u_d = nc.dram_tensor("u2", [128, 2], F32, kind="ExternalInput").ap()
    bv_d = nc.dram_tensor("bv", [1, H], F32, kind="ExternalInput").ap()
    out_d = nc.dram_tensor("out", [1, H], F32, kind="ExternalOutput").ap()

    with tile.TileContext(nc) as tc:
        with tc.tile_pool(name="const", bufs=1) as constp, \
             tc.tile_pool(name="big", bufs=1) as bigp, \
             tc.tile_pool(name="e", bufs=12) as ep, \
             tc.tile_pool(name="stat", bufs=6) as statp, \
             tc.tile_pool(name="rr", bufs=4) as rrp, \
             tc.tile_pool(name="wps", bufs=1, space="PSUM") as wpsp:

            # ---------- constants ----------
            m8 = constp.tile([128, 2, H], FP8, tag="m8")
            nc.scalar.dma_start(m8[:], m_d[:])
            wv16 = constp.tile([128, 2, H], F16, tag="wv16")
            nc.scalar.dma_start(wv16[:], wv_d[:])
            u2 = constp.tile([128, 2], F32, tag="u2")
            nc.scalar.dma_start(u2[:], u_d[:])
            bv = constp.tile([1, H], F32, tag="bv")
            nc.scalar.dma_start(bv[:], bv_d[:])
            ident = constp.tile([128, 128], F32, tag="ident")
            masks.make_identity(nc, ident[:])
            warm = constp.tile([1, 1], F32, tag="warm")
            nc.vector.memset(warm[:], 0.0)
            nc.scalar.activation(warm[:], warm[:], AF.Exp)

            # ---------- x (fp8, [128, half, n]) ----------
            x8 = bigp.tile([128, 2, N], FP8, tag="x8", name="x8")
            for half in range(2):
                for c in range(4):
                    eng = (nc.sync, nc.gpsimd)[c % 2]
                    eng.dma_start(
                        x8[:, half, c * 1024:(c + 1) * 1024],
                        x_d[:, half * N + c * 1024:half * N + (c + 1) * 1024])
            x16 = bigp.tile([128, 2, N], F16, tag="x16", name="x16")
            for half in range(2):
                for c in range(2):
                    nc.scalar.dma_start(
                        x16[:, half, c * 2048:(c + 1) * 2048],
                        x16_d[:, half * N + c * 2048:half * N + (c + 1) * 2048])

            qt8 = bigp.tile([128, 2, N], FP8, tag="qt8", name="qt8")
            v16 = bigp.tile([128, NQ * H], F16, tag="v16", name="v16")

            # ---------- phase 1: projections ----------
            with tc.tile_pool(name="pps", bufs=2, space="PSUM") as pps:
                for hc in range(2):
                    for nt in range(8):
                        sl = slice(nt * 512, (nt + 1) * 512)
                        psq = pps.tile([128, 512], F32, tag="projq")
                        nc.tensor.matmul(
                            psq[:], m8[:, :, hc * 128:(hc + 1) * 128],
                            x8[:, :, sl], start=True, stop=True, perf_mode=DR)
                        nc.scalar.activation(qt8[:, hc, sl], psq[:],
                                             AF.Identity,
                                             bias=u2[:, hc:hc + 1])
                for kv in range(16):
                    psv = pps.tile([128, 512], F32, tag="projv")
                    for sub in range(2):
                        kc = kv * 2 + sub
                        for half in range(2):
                            nc.tensor.matmul(
                                psv[:, sub * 256:(sub + 1) * 256],
                                x16[:, half, kc * 128:(kc + 1) * 128],
                                wv16[:, half, :],
                                start=(half == 0), stop=(half == 1))
                    nc.vector.tensor_copy(
                        v16[:, kv * 512:(kv + 1) * 512], psv[:])

            # ---------- phase 2: scores -> exp -> w accumulation ----------
            # w region jj (k in [jj*512,(jj+1)*512)):
            #   jj in 0..3  -> psum bank jj, partition 0   (fp8 DoubleRow)
            #   jj in 4..7  -> psum bank jj-4, partition 32 (f16, col strip)
            w_ps = [wpsp.tile([128, 512], F32, tag=f"w{i}", name=f"w{i}")
                    for i in range(4)]
            for i in range(4):
                nc.vector.memset(w_ps[i][:], 0.0)
            NPAIR = NQ // 2

            def emit_matvec(pair, rr8, e8, rr16s, e16s):
                for kt in range(2):
                    for j in range(2):
                        jj = kt * 2 + j
                        nc.tensor.matmul(
                            w_ps[jj][0:1, :],
                            rr8[:, :, 0:1],
                            e8[kt][:, :, j * 512:(j + 1) * 512],
                            start=(pair == 0), stop=(pair == NPAIR - 1),
                            perf_mode=DR, skip_group_check=True,
                            tile_position=(0, 0))
                for t in range(2):
                    for kt in range(2):
                        for j in range(2):
                            jj = kt * 2 + j
                            nc.tensor.matmul(
                                w_ps[jj][32:33, :],
                                rr16s[t][:],
                                e16s[t][kt][:, j * 512:(j + 1) * 512],
                                start=(pair == 0 and t == 0),
                                stop=(pair == NPAIR - 1 and t == 1),
                                skip_group_check=True,
                                tile_position=(0, 32))

            with tc.tile_pool(name="sps", bufs=2, space="PSUM") as sps, \
                 tc.tile_pool(name="e16p", bufs=10) as ep16, \
                 tc.tile_pool(name="rr16p", bufs=6) as rrp16:
                pending = []
                cur_e8 = None
                cur_rr8 = None
                pair_e16 = None
                pair_rr16 = None
                for qc in range(NQ):
                    par = qc % 2
                    # emit delayed matvecs (DELAY_QC=2: pair p at qc=2p+4)
                    while pending and qc - (2 * pending[0][0] + 1) >= 3:
                        emit_matvec(*pending.pop(0))
                    if par == 0:
                        cur_e8 = [
                            ep.tile([128, 2, KSUB], FP8, tag=f"e8_{kt}",
                                    name=f"e8_{kt}_{qc}")
                            for kt in range(2)]
                        cur_rr8 = rrp.tile([128, 2, 16], FP8, tag="rr8")
                        pair_e16 = []
                        pair_rr16 = []
                    e16 = [ep16.tile([128, KSUB], F16, tag=f"e16_{kt}",
                                     name=f"e16_{kt}_{qc}")
                           for kt in range(2)]
                    rr16 = rrp16.tile([128, 1], F16, tag="rr16")
                    pair_e16.append(e16)
                    pair_rr16.append(rr16)
                    stats = statp.tile([128, KT], F32, tag="stats")
                    for kt in range(KT):
                        psc = sps.tile([128, KSUB], F32, tag="s")
                        for half in range(2):
                            ksl = slice(kt * KSUB + half * 512,
                                        kt * KSUB + (half + 1) * 512)
                            nc.tensor.matmul(
                                psc[:, half * 512:(half + 1) * 512],
                                qt8[:, :, qc * 128:(qc + 1) * 128],
                                x8[:, :, ksl],
                                start=True, stop=True, perf_mode=DR)
                        if kt < 2:
                            out_ap = cur_e8[kt][:, par, :]
                        else:
                            out_ap = e16[kt - 2][:]
                        on_dve = (kt == 3) or (kt == 2 and qc % 4 == 3)
                        if on_dve:
                            nc.vector.tensor_scalar(
                                out_ap.bitcast(U16), psc[:],
                                T16_SCALE, T16_BIAS,
                                op0=ALU.mult, op1=ALU.add)
                            nc.vector.tensor_reduce(
                                stats[:, kt:kt + 1], out_ap,
                                axis=AX.X, op=ALU.add)
                        else:
                            nc.scalar.activation(
                                out_ap, psc[:], AF.Exp,
                                accum_out=stats[:, kt:kt + 1])
                    rsum = statp.tile([128, 1], F32, tag="rsum")
                    nc.vector.tensor_reduce(rsum[:], stats[:],
                                            axis=AX.X, op=ALU.add)
                    rinv = statp.tile([128, 1], F32, tag="rinv")
                    nc.vector.reciprocal(rinv[:], rsum[:])
                    nc.vector.tensor_scalar(
                        rr16[:], rinv[:], float(N), None, op0=ALU.mult)
                    nc.vector.tensor_copy(cur_rr8[:, par, 0:1], rr16[:])
                    if par == 1:
                        pending.append(
                            (qc // 2, cur_rr8, cur_e8, pair_rr16, pair_e16))
                for p in pending:
                    emit_matvec(*p)

            # ---------- phase 3: out = w^T V / (N*4096) + bv ----------
            with tc.tile_pool(name="fps", bufs=2, space="PSUM") as fps:
                # scale into SBUF (strips {0,32} of each bank hold data; the
                # rest is zeros from the memset)
                w_sc = bigp.tile([128, 2048], F32, tag="w_sc")
                for b in range(4):
                    nc.vector.tensor_scalar(
                        w_sc[:, b * 512:(b + 1) * 512], w_ps[b][:],
                        2.0 ** -12, None, op0=ALU.mult)
                wt = bigp.tile([128, NQ], F16, tag="wt")
                for b in range(4):
                    for u in range(4):
                        tp = fps.tile([128, 128], F32, tag="tp")
                        nc.tensor.transpose(
                            tp[:], w_sc[:, b * 512 + u * 128:
                                        b * 512 + (u + 1) * 128], ident[:])
                        # tp col 0 -> region b (wt col 4b+u);
                        # tp col 32 -> region 4+b (wt col 16+4b+u)
                        nc.vector.tensor_copy(
                            wt[:, 4 * b + u:4 * b + u + 17:16],
                            tp[:, 0:33:32])
                # exact global normalization: divide by sum(wt) so all
                # multiplicative biases in the E/r pipeline cancel
                wsum = statp.tile([128, 1], F32, tag="wsum")
                nc.vector.tensor_reduce(wsum[:], wt[:], axis=AX.X, op=ALU.add)
                tsum = statp.tile([128, 1], F32, tag="tsum")
                nc.gpsimd.partition_all_reduce(
                    tsum[:], wsum[:], channels=128,
                    reduce_op=bass_isa.ReduceOp.add)
                tinv = statp.tile([1, 1], F32, tag="tinv")
                nc.vector.reciprocal(tinv[:], tsum[0:1, :])
                out_ps = fps.tile([1, H], F32, tag="outp")
                for kc in range(NQ):
                    nc.tensor.matmul(out_ps[:], wt[:, kc:kc + 1],
                                     v16[:, kc * H:(kc + 1) * H],
                                     start=(kc == 0), stop=(kc == NQ - 1))
                out_sb = bigp.tile([1, H], F32, tag="out_sb")
                nc.vector.scalar_tensor_tensor(
                    out_sb[:], out_ps[:], tinv[:, 0:1], bv[:],
                    op0=ALU.mult, op1=ALU.add)
                nc.sync.dma_start(out_d[:], out_sb[:])

    nc.compile()
    return nc


def _get_program():
    if "nc" not in _CACHE:
        _CACHE["nc"] = _build_program()
    return _CACHE["nc"]


def _prep_inputs(x, Wq, bq, Wk, bk, Wv, bv):
    """Host-side prep: fp8 quantization + layout. Returns per-core in_maps."""
    import ml_dtypes

    FP8 = ml_dtypes.float8_e4m3
    x = np.asarray(x, dtype=np.float32)
    Wq = np.asarray(Wq, dtype=np.float32)
    Wk = np.asarray(Wk, dtype=np.float32)
    Wv = np.asarray(Wv, dtype=np.float32)
    bq = np.asarray(bq, dtype=np.float32)
    bv = np.asarray(bv, dtype=np.float32)

    M = (Wq @ Wk.T) / 16.0                      # [D, D]
    u = (bq @ Wk.T) / 16.0                      # [D]
    m8 = np.ascontiguousarray(
        M.reshape(2, 128, D).transpose(1, 0, 2)).astype(FP8).reshape(128, 2 * D)
    wv16 = np.ascontiguousarray(
        Wv.reshape(2, 128, H).transpose(1, 0, 2)).astype(np.float16
                                                         ).reshape(128, 2 * H)
    u2 = np.ascontiguousarray(u.reshape(2, 128).T)
    bv_row = np.ascontiguousarray(bv.reshape(1, H))

    in_maps = []
    for b in range(B):
        xt = np.ascontiguousarray(
            x[b].T.reshape(2, 128, N).transpose(1, 0, 2))   # [128, 2, N]
        x8 = xt.astype(FP8).reshape(128, 2 * N)
        x16 = xt.astype(np.float16).reshape(128, 2 * N)
        in_maps.append({
            "x8": x8, "x16": x16, "m8": m8, "wv16": wv16,
            "u2": u2, "bv": bv_row,
        })
    return in_maps


def kernel(x, Wq, bq, Wk, bk, Wv, bv):
    from concourse.bass_utils import run_bass_kernel_spmd

    nc = _get_program()
    in_maps = _prep_inputs(x, Wq, bq, Wk, bk, Wv, bv)
    res = run_bass_kernel_spmd(nc, in_maps, list(range(NCORES)))
    out = np.stack([res.results[b]["out"][0] for b in range(B)])
    return out.astype(np.float32)
